# revision 19
# baseline (speedup 1.0000x reference)
"""Trainium2 Bass kernel for nn_DiagnosticDeltaModel.

Model: h = embed[seq]; ff-MLP + residual + layernorm; k = hn @ Wkp;
DeltaNet-style scan  M <- M (I - kn kn^T) + k kn^T  over L-1 steps;
out = (M q @ Wrp + brp) @ Wout + bout.

Strategy: data-parallel over batch (4 batches per core x 8 cores).
The scan is parallelized with the chunked WY/UT transform: 128-token
spans, each split into four 32-token mini-chunks.  The four mini
triangular solves (I + tril(G))^-1 [K|V] are computed together as one
block-diagonal Neumann-product chain on [128,x] tiles (the strictly
triangular blocks are nilpotent; 4 factors are numerically exact for
32-token spans of this data).  Cross-mini state updates
S <- S + K_m^T (U_m - W_m S) run sequentially (S = M^T).

fp32 everywhere in the solve (PE fp32 is exact); float32r (12-bit
mantissa, full-rate at N>=256) only for the z1 MLP matmul and the
final Wout projection.
"""

import os
import sys

import numpy as np

if "/opt/trn_rl_repo" not in sys.path:
    sys.path.insert(0, "/opt/trn_rl_repo")

import concourse.bass as bass
import concourse.mybir as mybir
import concourse.tile as tile
from concourse import bacc
from concourse.bass import IndirectOffsetOnAxis
from concourse.bass_utils import run_bass_kernel_spmd
from concourse.masks import make_identity

H = 128
V = 32000
B = 32
L = 4096
NCORES = 8
BL = B // NCORES          # batches per core
NSPAN = L // 128          # 128-token spans per batch
NMINI = 4                 # 32-token minis per span
CH = 32
NFACT = 4                 # Neumann factors (N^1,2,4,8)
LN_EPS = 1e-5
NORM_EPS = 1e-12

f32 = mybir.dt.float32
f32r = mybir.dt.float32r
i32 = mybir.dt.int32
AX = mybir.AxisListType
OP = mybir.AluOpType
AF = mybir.ActivationFunctionType

# spans per batch actually emitted (for fast debug sims)
N_SPANS = int(os.environ.get("KERNEL_DEBUG_SPANS", NSPAN))
KP_NO_GATHER = bool(os.environ.get("KP_NO_GATHER"))
KP_NO_QMASK = bool(os.environ.get("KP_NO_QMASK"))
KP_NO_SCAN = bool(os.environ.get("KP_NO_SCAN"))
KP_NO_FINAL = bool(os.environ.get("KP_NO_FINAL"))
KP_NO_READOUT = bool(os.environ.get("KP_NO_READOUT"))

_PROGRAM = None


def _round12(x):
    """Round fp32 mantissa to 12 explicit bits (float32r's precision)."""
    m, e = np.frexp(np.asarray(x, np.float32))
    return (np.round(m * 4096.0) / 4096.0 * 2.0 ** e).astype(np.float32)


def _bcast_ap(handle, parts, offset, n):
    """DMA access pattern broadcasting a [1, n] dram row across `parts` partitions."""
    return bass.AP(handle, offset, [[0, parts], [1, n]])


def build_program():
    """Build and compile the per-core Bass program (identical on all cores)."""
    nc = bacc.Bacc(
        "TRN2",
        target_bir_lowering=False,
        debug=False,
        enable_asserts=False,
        num_devices=NCORES,
    )

    seqT_d = nc.dram_tensor("seqT", [128, BL * NSPAN], i32, kind="ExternalInput")
    embed_d = nc.dram_tensor("embed", [V, H], f32, kind="ExternalInput")
    w1_d = nc.dram_tensor("W1", [H, 2 * H], f32r, kind="ExternalInput")
    b1_d = nc.dram_tensor("b1", [1, 2 * H], f32, kind="ExternalInput")
    w2_d = nc.dram_tensor("W2", [2 * H, H], f32, kind="ExternalInput")
    b2_d = nc.dram_tensor("b2", [1, H], f32, kind="ExternalInput")
    wkp_d = nc.dram_tensor("Wkp", [H, H], f32, kind="ExternalInput")
    bk_d = nc.dram_tensor("bk", [1, H], f32, kind="ExternalInput")
    wrp_d = nc.dram_tensor("Wrp", [H, H], f32, kind="ExternalInput")
    brp_d = nc.dram_tensor("brp", [1, H], f32, kind="ExternalInput")
    wout_d = nc.dram_tensor("Wout", [H, V], f32r, kind="ExternalInput")
    bout_d = nc.dram_tensor("bout", [1, V], f32, kind="ExternalInput")
    maskl_d = nc.dram_tensor("maskL", [128, 128], f32, kind="ExternalInput")
    masku_d = nc.dram_tensor("maskU", [128, 128], f32, kind="ExternalInput")
    out_d = nc.dram_tensor("out", [BL, V], f32, kind="ExternalOutput")
    dbg_d = None
    if os.environ.get("KERNEL_DEBUG_DUMP"):
        dbg_d = nc.dram_tensor("dbg", [14, 128, 256], f32, kind="ExternalOutput")

    with tile.TileContext(nc) as tc:
        _emit(nc, tc, locals())
    nc.compile()
    return nc


def _emit(nc, tc, t):
    from contextlib import ExitStack

    with ExitStack() as ctx:
        const = ctx.enter_context(tc.tile_pool(name="const", bufs=1))
        gx = ctx.enter_context(tc.tile_pool(name="gx", bufs=3))
        sba = ctx.enter_context(tc.tile_pool(name="sba", bufs=3))
        xpool = ctx.enter_context(tc.tile_pool(name="xpool", bufs=3))
        chain = ctx.enter_context(tc.tile_pool(name="chain", bufs=2))
        fin = ctx.enter_context(tc.tile_pool(name="fin", bufs=4))
        ps_t = ctx.enter_context(tc.tile_pool(name="ps_t", bufs=2, space="PSUM"))
        ps_w = ctx.enter_context(tc.tile_pool(name="ps_w", bufs=2, space="PSUM"))
        ps_s = ctx.enter_context(tc.tile_pool(name="ps_s", bufs=3, space="PSUM"))

        # ---- constants / weights -------------------------------------------------
        ident = const.tile([128, 128], f32, tag="ident")
        make_identity(nc, ident[:])

        seqT = const.tile([128, BL * NSPAN], i32, tag="seqT")
        nc.sync.dma_start(out=seqT[:], in_=t["seqT_d"].ap())

        w1 = const.tile([H, 2 * H], f32r, tag="w1")
        nc.sync.dma_start(out=w1[:], in_=t["w1_d"].ap())
        w2a = const.tile([H, H], f32, tag="w2a")
        w2b = const.tile([H, H], f32, tag="w2b")
        nc.sync.dma_start(out=w2a[:], in_=t["w2_d"].ap()[0:128, :])
        nc.sync.dma_start(out=w2b[:], in_=t["w2_d"].ap()[128:256, :])
        wkp = const.tile([H, H], f32, tag="wkp")
        nc.sync.dma_start(out=wkp[:], in_=t["wkp_d"].ap())
        wrp = const.tile([H, H], f32, tag="wrp")
        nc.sync.dma_start(out=wrp[:], in_=t["wrp_d"].ap())

        b1B = const.tile([128, 2 * H], f32, tag="b1B")
        nc.sync.dma_start(out=b1B[:], in_=_bcast_ap(t["b1_d"], 128, 0, 2 * H))
        b2B = const.tile([128, H], f32, tag="b2B")
        nc.sync.dma_start(out=b2B[:], in_=_bcast_ap(t["b2_d"], 128, 0, H))
        bkB = const.tile([128, H], f32, tag="bkB")
        nc.sync.dma_start(out=bkB[:], in_=_bcast_ap(t["bk_d"], 128, 0, H))
        brpB = const.tile([BL, H], f32, tag="brpB")
        nc.sync.dma_start(out=brpB[:], in_=_bcast_ap(t["brp_d"], BL, 0, H))

        maskL = const.tile([128, 128], f32, tag="maskL")
        nc.sync.dma_start(out=maskL[:], in_=t["maskl_d"].ap())
        maskU = const.tile([128, 128], f32, tag="maskU")
        nc.sync.dma_start(out=maskU[:], in_=t["masku_d"].ap())

        epsln = const.tile([128, 1], f32, tag="epsln")
        nc.vector.memset(epsln[:], LN_EPS)

        # [32,1] mask: 1.0 on rows 0..30, 0.0 on row 31 (zeroes the query row)
        qmask = const.tile([CH, 1], f32, tag="qmask")
        nc.gpsimd.memset(qmask[:], 1.0)
        nc.gpsimd.affine_select(
            out=qmask[:],
            in_=qmask[:],
            compare_op=OP.is_ge,
            fill=0.0,
            base=CH - 2,
            pattern=[[0, 1]],
            channel_multiplier=-1,
        )

        wout = const.tile([H, V], f32r, tag="wout")
        for j in range(16):
            nc.sync.dma_start(
                out=wout[:, j * 2000:(j + 1) * 2000],
                in_=t["wout_d"].ap()[:, j * 2000:(j + 1) * 2000],
            )

        S = []
        for b in range(BL):
            sb = const.tile([H, H], f32, tag=f"S{b}")
            nc.vector.memset(sb[:], 0.0)
            S.append(sb)

        qrows = const.tile([BL, H], f32, tag="qrows")

        embed_ap = t["embed_d"].ap()

        # round-robin for psum->sbuf copies
        def cp(idx, out_ap, in_ap):
            if idx % 2 == 0:
                nc.scalar.copy(out_ap, in_ap)
            else:
                nc.vector.tensor_copy(out_ap, in_ap)

        # ---- one 128-token span of one batch ------------------------------------
        def span_body(b, s):
            col = b * NSPAN + s

            x = gx.tile([128, H], f32, tag="x")
            if KP_NO_GATHER:
                nc.vector.memset(x[:], 0.01)
            else:
                nc.gpsimd.indirect_dma_start(
                    out=x[:],
                    out_offset=None,
                    in_=embed_ap,
                    in_offset=IndirectOffsetOnAxis(ap=seqT[:, col:col + 1], axis=0),
                )

            # xT (f32r: feeds the z1 matmul only)
            xT_ps = ps_t.tile([128, 128], f32, tag="t")
            nc.tensor.transpose(xT_ps[:], x[:], ident[:])
            xT = sba.tile([128, 128], f32r, tag="xT")
            nc.scalar.copy(xT[:], xT_ps[:])

            # z1 = x @ W1   [tok, 2H]
            z1_ps = ps_w.tile([128, 2 * H], f32, tag="w")
            nc.tensor.matmul(z1_ps[:], lhsT=xT[:], rhs=w1[:], start=True, stop=True)
            z1b = sba.tile([128, 2 * H], f32, tag="z1b")
            nc.vector.tensor_add(z1b[:], z1_ps[:], b1B[:])
            r = sba.tile([128, 2 * H], f32, tag="r")
            nc.gpsimd.tensor_scalar_max(r[:], z1b[:], 0.0)

            # rT
            rT = sba.tile([128, 2 * H], f32, tag="rT")
            for hh in range(2):
                t_ps = ps_t.tile([128, 128], f32, tag="t")
                nc.tensor.transpose(t_ps[:], r[:, hh * 128:(hh + 1) * 128], ident[:])
                cp(hh, rT[:, hh * 128:(hh + 1) * 128], t_ps[:])

            # z2 = r @ W2   [tok, H]
            z2_ps = ps_s.tile([128, H], f32, tag="s")
            nc.tensor.matmul(z2_ps[:], lhsT=rT[:, 0:128], rhs=w2a[:], start=True, stop=False)
            nc.tensor.matmul(z2_ps[:], lhsT=rT[:, 128:256], rhs=w2b[:], start=False, stop=True)

            # h = x + z2 + b2 ; layernorm
            h = sba.tile([128, H], f32, tag="h")
            nc.vector.tensor_add(h[:], x[:], z2_ps[:])
            nc.gpsimd.tensor_tensor(h[:], h[:], b2B[:], op=OP.add)
            stats = sba.tile([128, 6], f32, tag="stats")
            nc.vector.bn_stats(stats[:], h[:])
            mv = sba.tile([128, 2], f32, tag="mv")
            nc.vector.bn_aggr(mv[:], stats[:])
            sd = sba.tile([128, 1], f32, tag="sd")
            nc.scalar.activation(sd[:], mv[:, 1:2], AF.Sqrt, bias=epsln[:])
            rinv = sba.tile([128, 1], f32, tag="rinv")
            nc.vector.reciprocal(rinv[:], sd[:])
            nmr = sba.tile([128, 1], f32, tag="nmr")
            nc.vector.tensor_tensor(nmr[:], mv[:, 0:1], rinv[:], op=OP.mult)
            nc.vector.tensor_scalar_mul(nmr[:], nmr[:], -1.0)
            hn = sba.tile([128, H], f32, tag="hn")
            nc.scalar.activation(hn[:], h[:], AF.Identity, bias=nmr[:], scale=rinv[:])

            # hnT ; k rows = hn @ Wkp + bk  -> X right half
            hnT_ps = ps_t.tile([128, 128], f32, tag="t")
            nc.tensor.transpose(hnT_ps[:], hn[:], ident[:])
            hnT = sba.tile([128, 128], f32, tag="hnT")
            nc.scalar.copy(hnT[:], hnT_ps[:])
            X = xpool.tile([128, 2 * H], f32, tag="X0")
            X0 = X
            k_ps = ps_s.tile([128, H], f32, tag="s")
            nc.tensor.matmul(k_ps[:], lhsT=hnT[:], rhs=wkp[:], start=True, stop=True)
            nc.vector.tensor_add(X[:, 128:256], k_ps[:], bkB[:])
            if t["dbg_d"] is not None and b == 0 and s == 0:
                nc.sync.dma_start(out=t["dbg_d"].ap()[6, :, 0:128], in_=hnT[:])
                nc.sync.dma_start(out=t["dbg_d"].ap()[7, :, 0:128], in_=X[:, 128:256])

            # kn rows = kraw / max(||kraw||, eps)  -> X left half
            sq = sba.tile([128, H], f32, tag="sq")
            nc.gpsimd.tensor_tensor(sq[:], X[:, 128:256], X[:, 128:256], op=OP.mult)
            ss = sba.tile([128, 1], f32, tag="ss")
            nc.vector.tensor_reduce(ss[:], sq[:], axis=AX.X, op=OP.add)
            nrm = sba.tile([128, 1], f32, tag="nrm")
            nc.scalar.sqrt(nrm[:], ss[:])
            nc.vector.tensor_scalar_max(nrm[:], nrm[:], NORM_EPS)
            rkn = sba.tile([128, 1], f32, tag="rkn")
            nc.vector.reciprocal(rkn[:], nrm[:])
            nc.vector.tensor_scalar_mul(X[:, 0:128], X[:, 128:256], rkn[:])

            if s == N_SPANS - 1 and not KP_NO_QMASK:
                # last row of the last span is the query, not a key: save it,
                # then zero it so it contributes nothing to the scan.
                nc.sync.dma_start(out=qrows[b:b + 1, :], in_=X[127:128, 128:256])
                nc.vector.tensor_scalar_mul(X[96:128, :], X[96:128, :], qmask[:, 0:1])

            if t["dbg_d"] is not None and b == 0 and s == 0:
                nc.sync.dma_start(out=t["dbg_d"].ap()[0, :, 0:128], in_=x[:])
                nc.sync.dma_start(out=t["dbg_d"].ap()[1, :, 0:128], in_=hn[:])
                nc.sync.dma_start(out=t["dbg_d"].ap()[2, :, :], in_=X[:])

            if KP_NO_SCAN:
                return
            # knT ; packed G = kn kn^T
            knT_ps = ps_t.tile([128, 128], f32, tag="t")
            nc.tensor.transpose(knT_ps[:], X[:, 0:128], ident[:])
            knT = sba.tile([128, 128], f32, tag="knT")
            nc.scalar.copy(knT[:], knT_ps[:])
            G_ps = ps_s.tile([128, 128], f32, tag="s")
            nc.tensor.matmul(G_ps[:], lhsT=knT[:], rhs=knT[:], start=True, stop=True)

            # block-diagonal strictly-triangular N and N^T (values are -G)
            cur = chain.tile([128, 128], f32, tag="npow")
            nc.vector.tensor_tensor(cur[:], G_ps[:], maskL[:], op=OP.mult)
            curT = chain.tile([128, 128], f32, tag="npowT")
            nc.vector.tensor_tensor(curT[:], G_ps[:], maskU[:], op=OP.mult)

            # X <- (I + N^8)(I + N^4)(I + N^2)(I + N) X, squaring as we go
            for lvl in range(NFACT):
                Xp = ps_w.tile([128, 2 * H], f32, tag="w")
                nc.tensor.matmul(Xp[:], lhsT=curT[:], rhs=X[:], start=True, stop=True)
                Xn = xpool.tile([128, 2 * H], f32, tag="X")
                nc.vector.tensor_add(Xn[:], X[:], Xp[:])
                X = Xn
                if lvl < NFACT - 1:
                    nq_ps = ps_s.tile([128, 128], f32, tag="s")
                    nc.tensor.matmul(nq_ps[:], lhsT=curT[:], rhs=cur[:], start=True, stop=True)
                    ncur = chain.tile([128, 128], f32, tag="npow")
                    nc.scalar.copy(ncur[:], nq_ps[:])
                    nqT_ps = ps_t.tile([128, 128], f32, tag="t")
                    nc.tensor.transpose(nqT_ps[:], ncur[:], ident[:])
                    ncurT = chain.tile([128, 128], f32, tag="npowT")
                    nc.vector.tensor_copy(ncurT[:], nqT_ps[:])
                    cur, curT = ncur, ncurT

            if t["dbg_d"] is not None and b == 0 and s == 0:
                nc.sync.dma_start(out=t["dbg_d"].ap()[3, :, :], in_=X[:])
                nc.sync.dma_start(out=t["dbg_d"].ap()[4, :, 0:128], in_=cur[:])

            # W^T for all minis at once
            wt_ps = ps_t.tile([128, 128], f32, tag="t")
            nc.tensor.transpose(wt_ps[:], X[:, 0:128], ident[:])
            wt = sba.tile([128, 128], f32, tag="wt")
            nc.scalar.copy(wt[:], wt_ps[:])

            # sequential mini updates: S_m = S_0 + sum_{j<=m} K_j^T Z_j with
            # Z_j = U_j - W_j S_{j-1}.  Zf rows outside processed minis stay
            # zero, so full K=128 matmuls accumulate exactly the right sum.
            Zf = sba.tile([128, 128], f32, tag="Z")
            nc.vector.memset(Zf[:], 0.0)
            scur = S[b]
            for m in range(NMINI):
                sl = slice(CH * m, CH * (m + 1))
                Yf = ps_s.tile([128, 128], f32, tag="s")
                nc.tensor.matmul(Yf[:], lhsT=wt[:], rhs=scur[:], start=True, stop=True)
                nc.vector.tensor_tensor(Zf[sl, :], X[sl, 128:256], Yf[sl, :], op=OP.subtract)
                P2_ps = ps_s.tile([128, 128], f32, tag="s")
                nc.tensor.matmul(P2_ps[:], lhsT=X0[:, 0:128], rhs=Zf[:], start=True, stop=True)
                if m < NMINI - 1:
                    snew = sba.tile([128, 128], f32, tag="Swork")
                else:
                    snew = S[b]
                nc.vector.tensor_add(snew[:], S[b][:], P2_ps[:])
                scur = snew
                if t["dbg_d"] is not None and b == 0 and s == 0:
                    nc.sync.dma_start(out=t["dbg_d"].ap()[10 + m, :, 0:128], in_=snew[:])
                    nc.sync.dma_start(out=t["dbg_d"].ap()[10 + m, :, 128:256], in_=Zf[:])

        for s in range(N_SPANS):
            for b in range(BL):
                span_body(b, s)
        if t["dbg_d"] is not None:
            nc.sync.dma_start(out=t["dbg_d"].ap()[5, :, 0:128], in_=S[0][:])

        if KP_NO_READOUT or KP_NO_SCAN:
            z512 = fin.tile([BL, 512], f32, tag="osb")
            nc.vector.memset(z512[:], 0.0)
            for j in range(V // 512):
                nc.sync.dma_start(out=t["out_d"].ap()[:, j * 512:(j + 1) * 512], in_=z512[:])
            nc.sync.dma_start(out=t["out_d"].ap()[:, V - 256:V], in_=z512[:, 0:256])
            return
        # ---- readout: read_b = S_b^T q_b (assembled as columns) -----------------
        qT_ps = ps_t.tile([128, BL], f32, tag="t")
        nc.tensor.transpose(qT_ps[:], qrows[:], ident[0:BL, 0:BL])
        qT = sba.tile([128, BL], f32, tag="qT")
        nc.scalar.copy(qT[:], qT_ps[:])
        readsT_ps = ps_s.tile([128, BL], f32, tag="s")
        for b in range(BL):
            nc.tensor.matmul(readsT_ps[:, b:b + 1], lhsT=S[b][:], rhs=qT[:, b:b + 1],
                             start=True, stop=True)
        readsT = sba.tile([128, BL], f32, tag="readsT")
        nc.scalar.copy(readsT[:], readsT_ps[:])

        # rp = reads @ Wrp + brp
        rp_ps = ps_s.tile([BL, H], f32, tag="s")
        nc.tensor.matmul(rp_ps[:], lhsT=readsT[:], rhs=wrp[:], start=True, stop=True)
        rp = sba.tile([BL, H], f32, tag="rp")
        nc.vector.tensor_add(rp[:], rp_ps[:], brpB[:])
        rpT_ps = ps_t.tile([128, BL], f32, tag="t")
        nc.tensor.transpose(rpT_ps[:], rp[:], ident[0:BL, 0:BL])
        rpT = sba.tile([128, BL], f32r, tag="rpT")
        nc.scalar.copy(rpT[:], rpT_ps[:])

        if t["dbg_d"] is not None:
            nc.sync.dma_start(out=t["dbg_d"].ap()[8, 0:BL, 0:128], in_=qrows[:])
            nc.sync.dma_start(out=t["dbg_d"].ap()[8, :, 128:128 + BL], in_=readsT[:])
            nc.sync.dma_start(out=t["dbg_d"].ap()[9, 0:BL, 0:128], in_=rp[:])
            nc.sync.dma_start(out=t["dbg_d"].ap()[9, :, 128:128 + BL], in_=rpT[:].bitcast(f32))

        # out = rp @ Wout + bout, streamed over 512-wide tiles
        ntiles = (V + 511) // 512
        for j in range(ntiles):
            lo = j * 512
            n = min(512, V - lo)
            bt = fin.tile([BL, 512], f32, tag="bt")
            nc.sync.dma_start(out=bt[:, 0:n], in_=_bcast_ap(t["bout_d"], BL, lo, n))
            ot_ps = ps_w.tile([BL, 512], f32, tag="w")
            nc.tensor.matmul(ot_ps[:, 0:n], lhsT=rpT[:], rhs=wout[:, lo:lo + n], start=True, stop=True)
            osb = fin.tile([BL, 512], f32, tag="osb")
            nc.vector.tensor_add(osb[:, 0:n], ot_ps[:, 0:n], bt[:, 0:n])
            nc.sync.dma_start(out=t["out_d"].ap()[:, lo:lo + n], in_=osb[:, 0:n])


def prep_inputs(inputs):
    """Host-side: shard/layout per-core input maps from the full input dict."""
    seq = np.asarray(inputs["seq"], np.int64).astype(np.int32)
    embed = np.asarray(inputs["embed"], np.float32)
    W1 = np.asarray(inputs["W1"], np.float32)
    b1 = np.asarray(inputs["b1"], np.float32)
    W2 = np.asarray(inputs["W2"], np.float32)
    b2 = np.asarray(inputs["b2"], np.float32)
    ln_g = np.asarray(inputs["ln_g"], np.float32)
    ln_b = np.asarray(inputs["ln_b"], np.float32)
    Wkp = np.asarray(inputs["Wkp"], np.float32)
    Wrp = np.asarray(inputs["Wrp"], np.float32)
    brp = np.asarray(inputs["brp"], np.float32)
    Wout = np.asarray(inputs["Wout"], np.float32)
    bout = np.asarray(inputs["bout"], np.float32)

    # fold layernorm affine into the key projection
    Wkp_eff = (ln_g[:, None] * Wkp).astype(np.float32)
    bk = (ln_b @ Wkp).astype(np.float32)[None, :]

    blk = np.kron(np.eye(4, dtype=np.float32), np.ones((CH, CH), np.float32))
    maskL = (-np.tril(np.ones((128, 128), np.float32), -1) * blk).astype(np.float32)
    maskU = (-np.triu(np.ones((128, 128), np.float32), 1) * blk).astype(np.float32)

    shared = {
        "embed": embed,
        "W1": _round12(W1),
        "b1": b1[None, :] if b1.ndim == 1 else b1,
        "W2": W2,
        "b2": b2[None, :] if b2.ndim == 1 else b2,
        "Wkp": Wkp_eff,
        "bk": bk,
        "Wrp": Wrp,
        "brp": brp[None, :] if brp.ndim == 1 else brp,
        "Wout": _round12(Wout),
        "bout": bout[None, :] if bout.ndim == 1 else bout,
        "maskL": maskL,
        "maskU": maskU,
    }
    in_maps = []
    for c in range(NCORES):
        sl = seq[c * BL:(c + 1) * BL]                       # [BL, L]
        seqT = sl.reshape(BL, NSPAN, 128).transpose(2, 0, 1).reshape(128, BL * NSPAN)
        in_maps.append({"seqT": np.ascontiguousarray(seqT), **shared})
    return in_maps


def kernel(**inputs):
    global _PROGRAM
    if _PROGRAM is None:
        _PROGRAM = build_program()
    in_maps = prep_inputs(inputs)
    res = run_bass_kernel_spmd(
        _PROGRAM, in_maps, core_ids=list(range(NCORES)), trace=False
    )
    out = np.concatenate([res.results[c]["out"] for c in range(NCORES)], axis=0)
    return out.astype(np.float32)
